# revision 1
# baseline (speedup 1.0000x reference)
"""Trainium2 Bass kernel for nn_Design2VecBase (GCN message passing).

Strategy: the entire GCN pipeline depends only on the graph index (B samples
share G=4 graphs), so the heavy per-graph work (4 layers of A @ (X @ W) with
A [2048, 2048]) is computed once per distinct graph: graph g on core g.
Each core also evaluates the cheap per-sample tail (masked mean + MLPs) for
ALL B=32 samples against its own graph; the host then selects row b from the
core that owns graph_indices[b]. A is uploaded pre-transposed (the PE
contracts over the partition dim) in bf16; per-graph A stays resident in SBUF
across all 4 layers so it is read from HBM exactly once.

fp32 operands and constants are packed into one [128, K] block so each
consumer depends on a single DMA; explicit DMA ordering (xs chunks + weights
-> A row-tiles -> tail-MLP block) keeps the critical path fed first.
"""

import os
import time

import numpy as np
import ml_dtypes

import concourse.bass as bass
from concourse import bacc
import concourse.mybir as mybir
from concourse.tile import TileContext, add_dep_helper
from concourse.bass_utils import run_bass_kernel_spmd

F32 = mybir.dt.float32
F32R = mybir.dt.float32r
BF16 = mybir.dt.bfloat16
AF = mybir.ActivationFunctionType
ALU = mybir.AluOpType
AX = mybir.AxisListType

G, N, F = 4, 2048, 144
B, H, M_TP, M_HID, NL, L = 32, 128, 192, 256, 1, 4
P = 128
NT = N // P        # 16 row-tiles of the graph
NCH = N // 512     # 4 moving-dim chunks

# ---- fp32 const-block column layout (widths multiples of 4 = 16B lines)
_OFF = {}
_cur = 0
for _name, _w in [
    # weight block (layer biases + transpose identity + ones)
    ("b0", 4), ("bs", L), ("ident", P), ("ones", P),
    # late block (masked-mean + MLP tail)
    ("mkt", NT * B), ("tpt0", B), ("tpt1", B),
    ("wtp1a", M_HID), ("wtp1b", M_HID), ("btp1", 4),
    ("wtp2a", M_HID), ("wtp2b", M_HID), ("btp2r", M_HID),
    ("wf1a", M_HID), ("wf1b", M_HID), ("wf1c", M_HID), ("bf1", 4),
    ("wf2", 4), ("bf2", 4),
]:
    _OFF[_name] = _cur
    _cur += _w
CBLK_K = _cur
_WBLK = slice(0, _OFF["mkt"])
_LATE = slice(_OFF["mkt"], CBLK_K)
# f32r block: w0 | w02 | per-chunk xsT (1024 each: [0:512] rows 0..127 of the
# chunk, [512:1024] rows 128..143)
XR_W0 = 0
XR_W02 = H
XR_XC = [2 * H + 1024 * c for c in range(NCH)]
XR_K = 2 * H + 1024 * NCH


def _c(name, w=None):
    o = _OFF[name]
    return slice(o, o + (w if w is not None else 1))


def _build_program():
    nc = bacc.Bacc("TRN2")

    at_d = nc.dram_tensor("at", [N, N], BF16, kind="ExternalInput")
    xr_d = nc.dram_tensor("xr", [P, XR_K], F32R, kind="ExternalInput")
    wsb_d = nc.dram_tensor("wsb", [P, L * H + P], BF16, kind="ExternalInput")
    cb_d = nc.dram_tensor("cb", [P, CBLK_K], F32, kind="ExternalInput")
    out_d = nc.dram_tensor("out", [B, NL], F32, kind="ExternalOutput")

    with TileContext(nc) as tc:
        with (
            tc.tile_pool(name="singles", bufs=1) as sg,
            tc.tile_pool(name="xw_pool", bufs=2) as xwp,
            tc.tile_pool(name="scratch", bufs=3) as sp,
        ):
            cb = sg.tile([P, CBLK_K], F32)
            xr_sb = sg.tile([P, XR_K], F32R)
            ws_sb = sg.tile([P, L * H + P], BF16)
            at_sb = sg.tile([P, NT, N], BF16)

            w0 = xr_sb[:, XR_W0:XR_W0 + H]
            w02 = xr_sb[0:F - P, XR_W02:XR_W02 + H]
            b0 = cb[:, _c("b0")]
            bs = cb[:, _c("bs", L)]
            ident = cb[:, _c("ident", P)]
            ident32 = cb[0:B, _OFF["ident"]:_OFF["ident"] + B]
            identB = ws_sb[:, L * H:L * H + P]  # bf16 identity
            ones_row = cb[0:1, _c("ones", P)]   # [1, P] of ones
            ones_col = cb[:, _c("ones")]        # [P, 1] of ones

            x0t_sb = sg.tile([P, N], BF16)      # X0.T (bf16), GCN loop entry
            x0tf_sb = sg.tile([P, N], F32)      # X0.T (f32), residual source
            x0n_sb = sg.tile([P, NT, H], F32)   # X0 natural, residual
            xt_a = sg.tile([P, N], BF16)
            xt_b = sg.tile([P, N], BF16)
            h3t_sb = sg.tile([P, N], BF16)      # last-layer pre-activation .T
            xsum_sb = sg.tile([P, NT, H], F32)  # softmax(h3) + X0, natural
            covs_sb = sg.tile([P, B], F32)      # covT (already 1/count-scaled)
            mks_sb = sg.tile([P, NT, B], F32)   # masksT * (1/count)
            rct_sb = sg.tile([1, B], F32)
            rcb_sb = sg.tile([P, B], F32)
            cnt_sb = sg.tile([B, 1], F32)
            rc_sb = sg.tile([B, 1], F32)
            tp1t_sb = sg.tile([P, 2, B], F32)
            tp2_sb = sg.tile([B, M_HID], F32)
            tp2t_sb = sg.tile([P, 2, B], F32)
            zf1t_sb = sg.tile([P, 2, B], F32)
            out_sb = sg.tile([B, NL], F32)

            rep_n = int(os.environ.get("KERNEL_REP", "1"))
            for rep in range(rep_n):
                # ---- input DMAs, explicitly priority-ordered:
                #      xs chunks + weights -> ws -> A row-tiles -> late block
                d_early = [nc.sync.dma_start(
                    out=xr_sb[:, 0:XR_XC[0] + 1024],
                    in_=xr_d[:, 0:XR_XC[0] + 1024])]
                d_early.append(nc.sync.dma_start(out=cb[:, _WBLK],
                                                 in_=cb_d[:, _WBLK]))
                d_early.append(nc.sync.dma_start(out=ws_sb, in_=wsb_d[:]))
                for c in range(1, NCH):
                    d_early.append(nc.sync.dma_start(
                        out=xr_sb[:, XR_XC[c]:XR_XC[c] + 1024],
                        in_=xr_d[:, XR_XC[c]:XR_XC[c] + 1024]))
                d_at = []
                for k in range(NT):
                    d = nc.sync.dma_start(out=at_sb[:, k, :],
                                          in_=at_d[k * P:(k + 1) * P, :])
                    # gate A only on the first xr chunk + ws (the rest of the
                    # xs chunks are small and can share bandwidth)
                    for e in d_early[:3]:
                        add_dep_helper(d.ins, e.ins, reason="dma priority")
                    d_at.append(d)
                d_late = nc.sync.dma_start(out=cb[:, _LATE], in_=cb_d[:, _LATE])
                for a in d_at:
                    add_dep_helper(d_late.ins, a.ins, reason="dma priority")

                with tc.tile_pool(name="psA", bufs=1, space="PSUM") as psA:
                    def s1(xt_src, xw_dst, i, jlist):
                        """step 1 of layer i for row-tiles jlist: XW nat tiles"""
                        for j in jlist:
                            js = slice(j * P, (j + 1) * P)
                            ps = psA.tile([P, H], F32, tag="psn", bufs=2,
                                          name=f"s1_{i}_{j}")
                            nc.tensor.matmul(ps, xt_src[:, js],
                                             ws_sb[:, i * H:(i + 1) * H],
                                             start=True, stop=True)
                            nc.vector.tensor_copy(out=xw_dst[:, j, :], in_=ps)

                    # ---- X0T = relu(W0.T @ xsT + b0) per 512-chunk, plus
                    #      X0 natural via PE transposes (exact f32)
                    for c in range(NCH):
                        cs = slice(c * 512, (c + 1) * 512)
                        xo = XR_XC[c]
                        ps = psA.tile([P, 512], F32, tag="ps1", bufs=2)
                        nc.tensor.matmul(ps, w0, xr_sb[:, xo:xo + 512],
                                         start=True, stop=False)
                        nc.tensor.matmul(ps, w02,
                                         xr_sb[0:F - P, xo + 512:xo + 1024],
                                         start=False, stop=True)
                        nc.scalar.activation(x0t_sb[:, cs], ps, AF.Relu, bias=b0)
                        nc.scalar.activation(x0tf_sb[:, cs], ps, AF.Relu, bias=b0)
                        for j in range(4 * c, 4 * c + 4):
                            js = slice(j * P, (j + 1) * P)
                            pst0 = psA.tile([P, H], F32, tag="psn", bufs=2)
                            nc.tensor.transpose(pst0, x0tf_sb[:, js], ident)
                            nc.vector.tensor_copy(out=x0n_sb[:, j, :], in_=pst0)

                    # ---- GCN layers: X_{i+1}.T = act(XW_i.T-contract A.T + b)
                    mko = _OFF["mkt"]
                    pcov = None
                    xw_cur = xwp.tile([P, NT, H], BF16, tag="xw", name="xw0")
                    s1(x0t_sb, xw_cur, 0, range(NT))
                    xt_cur = x0t_sb
                    for i in range(L):
                        ps2 = [psA.tile([P, 512], F32, tag=f"ps2_{c}", bufs=1,
                                        name=f"ps2_{i}_{c}") for c in range(NCH)]
                        xt_next = (h3t_sb if i == L - 1
                                   else (xt_a if i % 2 == 0 else xt_b))
                        func = AF.Identity if i == L - 1 else AF.Relu
                        xw_next = (xwp.tile([P, NT, H], BF16, tag="xw",
                                            name=f"xw{i + 1}")
                                   if i < L - 1 else None)
                        if i == 0:
                            # layer 0 is gated by the A DMA: k-outer so all
                            # chunks advance as each A row-tile lands
                            for k in range(NT):
                                for c in range(NCH):
                                    nc.tensor.matmul(
                                        ps2[c], xw_cur[:, k, :],
                                        at_sb[:, k, c * 512:(c + 1) * 512],
                                        start=(k == 0), stop=(k == NT - 1))
                        for c in range(NCH):
                            if i > 0:  # chunk-outer: drain c while c+1 runs
                                for k in range(NT):
                                    nc.tensor.matmul(
                                        ps2[c], xw_cur[:, k, :],
                                        at_sb[:, k, c * 512:(c + 1) * 512],
                                        start=(k == 0), stop=(k == NT - 1))
                            cs = slice(c * 512, (c + 1) * 512)
                            if i < L - 1:
                                nc.scalar.activation(xt_next[:, cs], ps2[c],
                                                     func, bias=bs[:, i:i + 1])
                                # next layer's step-1 for this chunk's tiles
                                s1(xt_next, xw_next, i + 1,
                                   range(4 * c, 4 * c + 4))
                            else:
                                # fused tail: per-tile drain + per-node
                                # softmax over h + residual + covT, per chunk
                                for j in range(4 * c, 4 * c + 4):
                                    js = slice(j * P, (j + 1) * P)
                                    nc.vector.tensor_scalar_add(
                                        out=xt_next[:, js],
                                        in0=ps2[c][:, (j - 4 * c) * P:
                                                   (j - 4 * c + 1) * P],
                                        scalar1=bs[:, i:i + 1])
                                    pst = psA.tile([P, P], BF16, tag="psn",
                                                   bufs=2)
                                    nc.tensor.transpose(pst, h3t_sb[:, js],
                                                        identB)
                                    negmax = sp.tile([P, 1], F32, tag="negmax")
                                    nc.vector.tensor_reduce(
                                        negmax, pst, axis=AX.X, op=ALU.max,
                                        negate=True)
                                    expt = sp.tile([P, P], F32, tag="expt")
                                    sume = sp.tile([P, 1], F32, tag="sume")
                                    nc.scalar.activation(expt, pst, AF.Exp,
                                                         bias=negmax,
                                                         accum_out=sume)
                                    rcpe = sp.tile([P, 1], F32, tag="rcpe")
                                    nc.vector.reciprocal(rcpe, sume)
                                    nc.vector.scalar_tensor_tensor(
                                        out=xsum_sb[:, j, :], in0=expt,
                                        scalar=rcpe, in1=x0n_sb[:, j, :],
                                        op0=ALU.mult, op1=ALU.add)
                                    if pcov is None:
                                        pcov = psA.tile([P, B], F32,
                                                        tag="ps1", bufs=2)
                                    nc.tensor.matmul(
                                        pcov, xsum_sb[:, j, :],
                                        mks_sb[:, j, :],
                                        start=(j == 0), stop=(j == NT - 1))
                        xt_cur = xt_next
                        xw_cur = xw_next
                        if i == 1:
                            # tp MLP + mask counts: independent of the GCN;
                            # slots into the PE stream here, out of the tail
                            for mi in range(2):
                                ms = slice(mi * P, (mi + 1) * P)
                                ptp = psA.tile([P, B], F32, tag="ps1", bufs=2)
                                nc.tensor.matmul(
                                    ptp, cb[:, _c("wtp1a", M_HID)][:, ms],
                                    cb[:, _c("tpt0", B)], start=True, stop=False)
                                nc.tensor.matmul(
                                    ptp, cb[0:M_TP - P, _c("wtp1b", M_HID)][:, ms],
                                    cb[0:M_TP - P, _c("tpt1", B)],
                                    start=False, stop=True)
                                nc.scalar.activation(
                                    tp1t_sb[:, mi, :], ptp, AF.Relu,
                                    bias=cb[:, _OFF["btp1"] + mi:
                                            _OFF["btp1"] + mi + 1])
                            ptp2 = psA.tile([B, M_HID], F32, tag="ps1", bufs=2)
                            nc.tensor.matmul(ptp2, tp1t_sb[:, 0, :],
                                             cb[:, _c("wtp2a", M_HID)],
                                             start=True, stop=False)
                            nc.tensor.matmul(ptp2, tp1t_sb[:, 1, :],
                                             cb[:, _c("wtp2b", M_HID)],
                                             start=False, stop=False)
                            nc.tensor.matmul(ptp2, ones_row[:, 0:B],
                                             cb[0:1, _c("btp2r", M_HID)],
                                             start=False, stop=True)
                            nm2 = sp.tile([B, 1], F32, tag="nm2")
                            nc.vector.tensor_reduce(nm2, ptp2, axis=AX.X,
                                                    op=ALU.max, negate=True)
                            ex2 = sp.tile([B, M_HID], F32, tag="ex2")
                            se2 = sp.tile([B, 1], F32, tag="se2")
                            nc.scalar.activation(ex2, ptp2, AF.Exp, bias=nm2,
                                                 accum_out=se2)
                            rc2 = sp.tile([B, 1], F32, tag="rc2")
                            nc.vector.reciprocal(rc2, se2)
                            nc.scalar.mul(tp2_sb, ex2, rc2)
                            for mi in range(2):
                                ptt = psA.tile([P, B], F32, tag="ps1", bufs=2)
                                nc.tensor.transpose(
                                    ptt, tp2_sb[:, mi * P:(mi + 1) * P], ident32)
                                nc.scalar.copy(tp2t_sb[:, mi, :], ptt)
                            pcnt = psA.tile([B, 1], F32, tag="ps1", bufs=2)
                            for j in range(NT):
                                nc.tensor.matmul(
                                    pcnt, cb[:, mko + j * B:mko + (j + 1) * B],
                                    ones_col, start=(j == 0), stop=(j == NT - 1))
                            nc.vector.tensor_scalar_max(cnt_sb, pcnt, 1.0)
                            nc.vector.reciprocal(rc_sb, cnt_sb)
                            prt = psA.tile([1, B], F32, tag="psn", bufs=2)
                            nc.tensor.transpose(prt, rc_sb, ident32)
                            nc.scalar.copy(rct_sb, prt)
                            prb = psA.tile([P, B], F32, tag="psn", bufs=2)
                            nc.tensor.matmul(prb, ones_row, rct_sb,
                                             start=True, stop=True)
                            nc.scalar.copy(rcb_sb, prb)
                            for j in range(NT):
                                nc.vector.tensor_mul(
                                    out=mks_sb[:, j, :],
                                    in0=cb[:, mko + j * B:mko + (j + 1) * B],
                                    in1=rcb_sb)

                    # ---- cov already 1/count-scaled via the masks
                    nc.vector.tensor_copy(out=covs_sb, in_=pcov)

                    # ---- zf1T = relu(Wf1.T @ [covT_s; tp2T] + bf1)
                    for mi in range(2):
                        ms = slice(mi * P, (mi + 1) * P)
                        pz = psA.tile([P, B], F32, tag="ps1", bufs=2)
                        nc.tensor.matmul(pz, cb[:, _c("wf1b", M_HID)][:, ms],
                                         tp2t_sb[:, 0, :], start=True, stop=False)
                        nc.tensor.matmul(pz, cb[:, _c("wf1c", M_HID)][:, ms],
                                         tp2t_sb[:, 1, :], start=False, stop=False)
                        nc.tensor.matmul(pz, cb[:, _c("wf1a", M_HID)][:, ms],
                                         covs_sb, start=False, stop=True)
                        nc.scalar.activation(
                            zf1t_sb[:, mi, :], pz, AF.Relu,
                            bias=cb[:, _OFF["bf1"] + mi:_OFF["bf1"] + mi + 1])
                    # ---- out = sigmoid(zf1 @ Wf2 + bf2)
                    po = psA.tile([B, NL], F32, tag="psn", bufs=2)
                    nc.tensor.matmul(po, zf1t_sb[:, 0, :], cb[:, _c("wf2")],
                                     start=True, stop=False)
                    nc.tensor.matmul(po, zf1t_sb[:, 1, :],
                                     cb[:, _OFF["wf2"] + 1:_OFF["wf2"] + 2],
                                     start=False, stop=False)
                    nc.tensor.matmul(po, ones_row[:, 0:B], cb[0:1, _c("bf2")],
                                     start=False, stop=True)
                    # sigmoid via 1/(1+exp(-z)): keeps ACT on the Exp func
                    # table (no Sigmoid set exists alongside Exp, and the
                    # table reload would cost ~1.3us on the critical path)
                    eneg = sp.tile([B, NL], F32, tag="eneg")
                    nc.scalar.activation(eneg, po, AF.Exp, scale=-1.0)
                    ep1 = sp.tile([B, NL], F32, tag="ep1")
                    nc.vector.tensor_scalar_add(out=ep1, in0=eneg, scalar1=1.0)
                    nc.vector.reciprocal(out_sb, ep1)
                    nc.sync.dma_start(out=out_d[:], in_=out_sb)

    return nc


_NC = None


def _get_program():
    global _NC
    if _NC is None:
        _NC = _build_program()
        # Bacc.finalize() runs compile(): splits multi-sem waits into event
        # semaphores (walrus allows one sync-wait per instruction) and moves
        # matmul weight waits onto LDWEIGHTS. The pjrt exec path serializes
        # nc as-is, so finalize must happen here.
        _NC.finalize()
    return _NC


def _const_block(inputs) -> np.ndarray:
    cb = np.zeros((P, CBLK_K), np.float32)
    f32 = lambda x: np.asarray(x, dtype=np.float32)

    mk = f32(np.asarray(inputs["cp_masks"])).T           # [N, B]
    mko = _OFF["mkt"]
    for j in range(NT):
        cb[:, mko + j * B:mko + (j + 1) * B] = mk[j * P:(j + 1) * P]
    tp = f32(inputs["tps"]).T                            # [M_TP, B]
    cb[:, _c("tpt0", B)] = tp[0:P]
    cb[0:M_TP - P, _c("tpt1", B)] = tp[P:M_TP]
    cb[:, _c("b0")] = f32(inputs["b0"]).reshape(P, 1)
    cb[:, _c("bs", L)] = f32(inputs["gcn_bs"]).T
    wtp1 = f32(inputs["Wtp1"])
    cb[:, _c("wtp1a", M_HID)] = wtp1[0:P]
    cb[0:M_TP - P, _c("wtp1b", M_HID)] = wtp1[P:M_TP]
    cb[:, _OFF["btp1"]:_OFF["btp1"] + 2] = f32(inputs["btp1"]).reshape(2, P).T
    wtp2 = f32(inputs["Wtp2"])
    cb[:, _c("wtp2a", M_HID)] = wtp2[0:P]
    cb[:, _c("wtp2b", M_HID)] = wtp2[P:M_HID]
    cb[0:1, _c("btp2r", M_HID)] = f32(inputs["btp2"]).reshape(1, M_HID)
    wf1 = f32(inputs["Wf1"])
    cb[:, _c("wf1a", M_HID)] = wf1[0:P]
    cb[:, _c("wf1b", M_HID)] = wf1[P:2 * P]
    cb[:, _c("wf1c", M_HID)] = wf1[2 * P:3 * P]
    cb[:, _OFF["bf1"]:_OFF["bf1"] + 2] = f32(inputs["bf1"]).reshape(2, P).T
    wf2 = f32(inputs["Wf2"]).reshape(M_HID, NL)
    cb[:, _OFF["wf2"]:_OFF["wf2"] + 1] = wf2[0:P]
    cb[:, _OFF["wf2"] + 1:_OFF["wf2"] + 2] = wf2[P:M_HID]
    cb[0:1, _c("bf2")] = f32(inputs["bf2"]).reshape(1, 1)
    cb[:, _c("ident", P)] = np.eye(P, dtype=np.float32)
    cb[:, _c("ones", P)] = 1.0
    return cb


def _xr_block(inputs, xsT) -> np.ndarray:
    xr = np.zeros((P, XR_K), np.float32)
    w0 = np.asarray(inputs["W0"], dtype=np.float32)
    xr[:, XR_W0:XR_W0 + H] = w0[0:P]
    xr[0:F - P, XR_W02:XR_W02 + H] = w0[P:F]
    for c in range(NCH):
        xo = XR_XC[c]
        xr[:, xo:xo + 512] = xsT[0:P, c * 512:(c + 1) * 512]
        xr[0:F - P, xo + 512:xo + 1024] = xsT[P:F, c * 512:(c + 1) * 512]
    return xr


def _prep_in_maps(inputs) -> list:
    bf = lambda x: np.ascontiguousarray(
        np.asarray(x, dtype=np.float32).astype(ml_dtypes.bfloat16))
    ws = np.asarray(inputs["gcn_Ws"], dtype=np.float32)   # [L, H, H]
    wsb = np.concatenate([ws.transpose(1, 0, 2).reshape(P, L * H),
                          np.eye(P, dtype=np.float32)], axis=1)
    wsb = bf(wsb)                                         # [p, i*H+h | ident]
    gxs = np.asarray(inputs["graph_xs"])
    gas = np.asarray(inputs["graph_as"])
    cbk = _const_block(inputs)
    in_maps = []
    for g in range(G):
        xsT = np.ascontiguousarray(np.asarray(gxs[g]).T).astype(np.float32)
        in_maps.append({
            "at": bf(np.asarray(gas[g]).T),
            "xr": _xr_block(inputs, xsT),
            "wsb": wsb,
            "cb": cbk,
        })
    return in_maps


def kernel(**inputs) -> np.ndarray:
    nc = _get_program()
    idx = np.asarray(inputs["graph_indices"]).reshape(B).astype(np.int64)
    in_maps = _prep_in_maps(inputs)
    # first-touch launches occasionally hit transient NRT device errors after
    # a prior process crashed mid-run; a retry has always succeeded
    last = None
    for _attempt in range(3):
        try:
            res = run_bass_kernel_spmd(nc, in_maps, core_ids=list(range(G)))
            break
        except Exception as e:
            last = e
            time.sleep(2.0 * (_attempt + 1))
    else:
        raise last
    out = np.zeros((B, NL), np.float32)
    for b in range(B):
        out[b] = res.results[int(idx[b])]["out"][b]
    return out

